# revision 49
# baseline (speedup 1.0000x reference)
"""Trainium2 Bass kernel for the CRF loss (nn_CRFLayer).

Full-input contract: kernel(**inputs) takes the full [1024,512,32] emissions,
[1024,512] tags, [1024,512] mask (all-ones by construction), [32,32]
transitions; returns the scalar f32 loss.

Strategy (8 NeuronCores, data-parallel over batch, 128 rows/core):
  - Exp-space forward algorithm:  q_t = (E~^T q_{t-1}) * exp(em_t - 1/2),
    with E~ = exp(transitions) * exp(-a) (a: global scale fold, corrected
    exactly on the host afterwards).
  - Bidirectional split: a forward chain covers t=1..256, an independent
    backward chain covers t=511..257; they stitch at t=256 via
    Z_b = sum_i q_256[i,b] * rho_256[i,b], halving the serial depth.
  - Layout: state [128 partitions = (4 batch-groups x 32 tags), 32 free =
    batch-in-group].  The K=32 contraction is one 128x128 block-diagonal
    bf16 matmul (kron(I4, E~)) per direction, both into one PSUM bank.
  - The emissions arrive TWICE from the host, bf16: raw [b,(t,k)] layout
    (feeds the gold-score matmuls) and pre-transposed slot-ordered layout
    (feeds the chain), so no on-device transposes exist at all.  ACT
    exponentiates slot blocks into the "pairs" buffer one superstep-group
    ahead of use; a single DVE multiply per superstep advances both
    chains.  DVE runs nothing else (the critical PE->DVE->PE loop stays
    clean: Tile lowers semaphore waits as stream-position snapshots, so
    any engine work scheduled into a chain wait window would add its own
    execution time to the chain).
  - Each multiply writes a dedicated SBUF buffer (no pool recycling), so
    its single hardware wait slot carries the PE semaphore inline instead
    of an extra EventSemaphore instruction.
  - Renormalization (every 64 steps) runs OFF the critical chain: the
    per-batch reciprocal-of-sums is multiplied into the pairs buffer 12
    supersteps ahead (linearity makes deferred rescaling exact); the raw
    sums ship to the host, which applies exact log corrections.
  - Gold path score via one-hot pieces OH_c[b,t,j] = (tags[b,t]==j)
    (GPSIMD is_equal with broadcast APs, one [128,32*32] piece per chunk;
    GPSIMD is otherwise idle and the chain never waits on it):
      * emission score  = trace(M_em),  M_em = sum_{b,t} em x OH
      * transition score = sum(M_tr * transitions), M_tr = sum OH_t x OH_{t+1}
    both as long PSUM-accumulating PE matmul chains, dosed 5 matmuls per
    superstep (gated a few supersteps past their one-hot) to fill PE idle
    slots without queueing ahead of chain work.
  - Per-core output [4, 226] f32: per-batch stitch sums Z, em/tr gold
    scalars, and the 3 renorm sum blocks; host does the logs, the
    subtraction, the global scale correction, and the mean.
"""

import math
import numpy as np

B, T, K = 1024, 512, 32
NCORES = 8
BSH = B // NCORES          # 128 batch rows per core
G = 4                      # batch groups stacked on partitions
BG = BSH // G              # 32 batch per group
TAU = 256                  # stitch point
NORM_EVERY = 64
N_RENORM = TAU // NORM_EVERY - 1   # 3
CHUNK_T = 32               # timesteps per raw chunk ([128, 1024])
NCHUNKS = T // CHUNK_T     # 16
SS_GROUP = 32              # supersteps per group
N_GROUPS = TAU // SS_GROUP # 8
EXP_BIAS = -0.5            # em~ = exp(em + EXP_BIAS)
GOLD_DOSE = 5              # gold-score matmuls emitted per superstep
OH_DELAY = 6               # supersteps between one-hot emission and its golds

# const pack layouts
PBF_WF, PBF_WB, PBF_OBD, PBF_IOTA = 0, 128, 256, 260
PBF_COLS = 292
PF_OBD, PF_OBC, PF_EYE, PF_TRANS = 0, 4, 132, 164
PF_COLS = 196
# output layout: [4, OUT_COLS]
OC_Z, OC_EM, OC_TR, OC_SPS = 0, 32, 33, 34
OUT_COLS = OC_SPS + N_RENORM * 2 * BG   # 226

EMT_COLS = TAU * 2 * BG    # slot-ordered emissions, [128, 16384]
EM01_COLS = 2 * BG         # [em0 | em511]

# scheduler release-time estimate for superstep s (ms for tile_wait_until)
HEAD_NS = 5000
PERIOD_NS = 600


def _EST(s):
    return (HEAD_NS + PERIOD_NS * s) / 1e6

_PROGRAM_CACHE = {}


def _build_program():
    """Builds the single-core SPMD bass program."""
    import concourse.bass as bass
    import concourse.mybir as mybir
    import concourse.bacc as bacc
    from concourse import tile
    from concourse.bass_types import AP

    dt = mybir.dt
    AF = mybir.ActivationFunctionType
    OP = mybir.AluOpType

    nc = bacc.Bacc("TRN2", target_bir_lowering=False, debug=False)

    em_d = nc.declare_dram_parameter("emissions", [BSH, T, K], dt.bfloat16, isOutput=False)
    emt_d = nc.declare_dram_parameter("emt", [128, EMT_COLS], dt.bfloat16, isOutput=False)
    em01_d = nc.declare_dram_parameter("em01", [128, EM01_COLS], dt.bfloat16, isOutput=False)
    oh_d = nc.declare_dram_parameter("oh", [BSH, T * K], dt.bfloat16, isOutput=False)
    packbf_d = nc.declare_dram_parameter("packbf", [128, PBF_COLS], dt.bfloat16, isOutput=False)
    packf_d = nc.declare_dram_parameter("packf", [128, PF_COLS], dt.float32, isOutput=False)
    out_d = nc.declare_dram_parameter("out", [G, OUT_COLS], dt.float32, isOutput=True)

    with tile.TileContext(nc) as tc:
        with (
            tc.tile_pool(name="const", bufs=1) as constp,
            tc.tile_pool(name="rawF", bufs=4) as rawFp,
            tc.tile_pool(name="rawB", bufs=4) as rawBp,
            tc.tile_pool(name="misc", bufs=2) as miscp,
            tc.tile_pool(name="psP", bufs=3, space="PSUM") as psPp,
            tc.tile_pool(name="psN", bufs=1, space="PSUM") as psNp,
            tc.tile_pool(name="psZ", bufs=1, space="PSUM") as psZp,
            tc.tile_pool(name="psME", bufs=1, space="PSUM") as psMEp,
            tc.tile_pool(name="psMT", bufs=1, space="PSUM") as psMTp,
        ):
            # ---- head-critical DMAs, all on SP's (fast) queue in
            # criticality order; ACT's sequencer does only exps ----
            EMT_GRP = SS_GROUP * 2 * BG     # 2048 cols per superstep-group
            EMT_HEAD = 8 * 2 * BG           # first 8 slots land first
            em01 = constp.tile([128, EM01_COLS], dt.bfloat16)
            nc.sync.dma_start(out=em01[:], in_=em01_d[:])
            emt = constp.tile([128, EMT_COLS], dt.bfloat16)
            nc.sync.dma_start(out=emt[:, 0:EMT_HEAD], in_=emt_d[:, 0:EMT_HEAD])
            packbf = constp.tile([128, PBF_COLS], dt.bfloat16)
            nc.sync.dma_start(out=packbf[:], in_=packbf_d[:])
            nc.sync.dma_start(out=emt[:, EMT_HEAD:EMT_GRP],
                              in_=emt_d[:, EMT_HEAD:EMT_GRP])

            raw0 = rawFp.tile([128, CHUNK_T * K], dt.bfloat16)
            nc.sync.dma_start(out=raw0[:], in_=em_d[:, 0:CHUNK_T, :])
            raw15 = rawBp.tile([128, CHUNK_T * K], dt.bfloat16)
            nc.sync.dma_start(
                out=raw15[:], in_=em_d[:, (NCHUNKS - 1) * CHUNK_T:, :])

            packf = constp.tile([128, PF_COLS], dt.float32)
            oh_sb = constp.tile([BSH, T * K], dt.bfloat16)

            wf = packbf[:, PBF_WF:PBF_WF + 128]
            wb = packbf[:, PBF_WB:PBF_WB + 128]
            onesbd_bf = packbf[:, PBF_OBD:PBF_OBD + G]
            iota32 = packbf[:, PBF_IOTA:PBF_IOTA + K]
            onesbd_f = packf[:, PF_OBD:PF_OBD + G]
            onesbc_f = packf[0:G, PF_OBC:PF_OBC + 128]
            eye32 = packf[0:K, PF_EYE:PF_EYE + K]
            trans_sb = packf[0:K, PF_TRANS:PF_TRANS + K]

            expbias = constp.tile([128, 1], dt.float32)
            nc.vector.memset(expbias[:], EXP_BIAS)
            ones32 = constp.tile([K, 1], dt.float32)
            nc.vector.memset(ones32[:], 1.0)
            # dummy activation: hoists the 1.3us ACT table load to t~=1us,
            # off the head-critical path (it would otherwise run right
            # before the first real exp, after its input DMA lands)
            atl = constp.tile([1, 1], dt.float32)
            nc.scalar.activation(out=atl[:], in_=expbias[0:1, :],
                                 func=AF.Exp)

            out_sb = constp.tile([G, OUT_COLS], dt.float32)
            nc.vector.memset(out_sb[:], 0.0)

            # em~ pairs buffer: slot s = [fwd em~_{s+1} | bwd em~_{510-s}]
            pairs = constp.tile([128, TAU * 2 * BG], dt.bfloat16)
            em0 = constp.tile([128, BG], dt.bfloat16)
            em511 = constp.tile([128, BG], dt.bfloat16)

            oh_pieces = [None] * NCHUNKS
            raw_tiles = {0: raw0, 15: raw15}
            m_em = psMEp.tile([K, K], dt.float32, tag="m_em")
            m_tr = psMTp.tile([K, K], dt.float32, tag="m_tr")

            # ---- gold-score machinery (lazy, dosed) ----
            pending_gold = []   # heap-ish FIFO of (min_superstep, closure)
            mem_count = [0]
            mtr_count = [0]
            N_MEM = T
            N_MTR = T - 1
            cur_s = [0]

            def gold_mem(c, k):
                def emit():
                    nc.tensor.matmul(
                        out=m_em[:],
                        lhsT=raw_tiles[c][:, k * K:(k + 1) * K],
                        rhs=oh_pieces[c][:, k * K:(k + 1) * K],
                        start=(mem_count[0] == 0),
                        stop=(mem_count[0] == N_MEM - 1),
                        skip_group_check=True)
                    mem_count[0] += 1
                return emit

            def gold_mtr(t):
                def emit():
                    ca, sa = t // CHUNK_T, t % CHUNK_T
                    cb, sb = (t + 1) // CHUNK_T, (t + 1) % CHUNK_T
                    nc.tensor.matmul(
                        out=m_tr[:],
                        lhsT=oh_pieces[ca][:, sa * K:(sa + 1) * K],
                        rhs=oh_pieces[cb][:, sb * K:(sb + 1) * K],
                        start=(mtr_count[0] == 0),
                        stop=(mtr_count[0] == N_MTR - 1),
                        skip_group_check=True)
                    mtr_count[0] += 1
                return emit

            oh_emit_s = {}

            def prep_oh(c):
                # one-hots are host-built and DMA'd (no legal on-device
                # engine computes is_equal without touching DVE/ACT, whose
                # snapshot semaphores would couple it to the chain)
                nc.sync.dma_start(
                    out=oh_sb[:, c * CHUNK_T * K:(c + 1) * CHUNK_T * K],
                    in_=oh_d[:, c * CHUNK_T * K:(c + 1) * CHUNK_T * K])
                oh_pieces[c] = oh_sb[:, c * CHUNK_T * K:(c + 1) * CHUNK_T * K]
                oh_emit_s[c] = cur_s[0]
                ready = cur_s[0] + OH_DELAY
                for k in range(CHUNK_T):
                    pending_gold.append((ready, gold_mem(c, k)))

            def queue_mtr_chunk(c):
                # both one-hots of every t in this range exist already
                ready = max(oh_emit_s.get(c, 0),
                            oh_emit_s.get(c + 1, 0)) + OH_DELAY
                lo = c * CHUNK_T
                hi = min((c + 1) * CHUNK_T, T - 1)
                for t in range(lo, hi):
                    pending_gold.append((ready, gold_mtr(t)))

            def pop_gold(s):
                n = 0
                while n < GOLD_DOSE and pending_gold and pending_gold[0][0] <= s:
                    pending_gold.pop(0)[1]()
                    n += 1

            # ---- pairs exp: slot blocks from the pre-transposed emt ----
            def exp_slots(lo, n):
                nc.scalar.activation(
                    out=pairs[:, lo * 2 * BG:(lo + n) * 2 * BG],
                    in_=emt[:, lo * 2 * BG:(lo + n) * 2 * BG],
                    func=AF.Exp, bias=expbias[:])

            # ---- chain state ----
            st = {"q_rhs": em0[:], "v_rhs": em511[:], "rho": None}
            renorm_idx = [0]

            def superstep(s):
                ps = psPp.tile([128, 2 * BG], dt.float32, tag="qv")
                nc.tensor.matmul(out=ps[:, 0:BG], lhsT=wf, rhs=st["q_rhs"],
                                 start=True, stop=True)
                if s <= 254:
                    nc.tensor.matmul(out=ps[:, BG:2 * BG], lhsT=wb,
                                     rhs=st["v_rhs"], start=True, stop=True)
                if s <= 253:
                    # dedicated buffer per superstep: no pool WAW self-wait,
                    # so the mult's single wait slot holds the PE sem inline
                    qv = constp.tile([128, 2 * BG], dt.bfloat16, tag=f"qv{s}")
                    nc.vector.tensor_tensor(
                        out=qv[:], in0=ps[:],
                        in1=pairs[:, 2 * BG * s:2 * BG * (s + 1)], op=OP.mult)
                    st["q_rhs"] = qv[:, 0:BG]
                    st["v_rhs"] = qv[:, BG:2 * BG]
                    if (s + 1) % NORM_EVERY == 0 and s + 1 < TAU:
                        # off-chain renorm: rescale pairs slot s+12 by
                        # 1/sums; raw sums ship to host for exact log fixup.
                        # Each sub-op is released into a distinct later
                        # window so no chain wait snapshots a long residue.
                        r = renorm_idx[0]
                        renorm_idx[0] += 1
                        s_ps = psNp.tile([G, 2 * BG], dt.float32, tag="s_ps")
                        nc.tensor.matmul(out=s_ps[:], lhsT=onesbd_bf,
                                         rhs=qv[:], start=True, stop=True)
                        nc.vector.tensor_copy(
                            out=out_sb[:, OC_SPS + r * 2 * BG:
                                       OC_SPS + (r + 1) * 2 * BG],
                            in_=s_ps[:])
                        rs = miscp.tile([G, 2 * BG], dt.float32, tag="rs")
                        nc.vector.reciprocal(out=rs[:], in_=s_ps[:])
                        bc_ps = psNp.tile([128, 2 * BG], dt.float32, tag="bc_ps")
                        nc.tensor.matmul(out=bc_ps[:], lhsT=onesbc_f,
                                         rhs=rs[:], start=True, stop=True)
                        sl = pairs[:, 2 * BG * (s + 12):2 * BG * (s + 13)]
                        nc.vector.tensor_tensor(out=sl, in0=sl, in1=bc_ps[:],
                                                op=OP.mult)
                else:
                    qv = constp.tile([128, BG], dt.bfloat16, tag=f"qv{s}")
                    nc.vector.tensor_tensor(
                        out=qv[:], in0=ps[:, 0:BG],
                        in1=pairs[:, 2 * BG * s:2 * BG * s + BG], op=OP.mult)
                    st["q_rhs"] = qv[:]
                    if s == 254:
                        st["rho"] = ps

            def emit_gold_finalize():
                # em gold: trace(m_em);  tr gold: sum(m_tr * transitions).
                me = miscp.tile([K, K], dt.float32, tag="me")
                nc.vector.tensor_tensor(out=me[:], in0=m_em[:], in1=eye32,
                                        op=OP.mult)
                me_r = miscp.tile([K, 1], dt.float32, tag="me_r")
                nc.vector.tensor_reduce(out=me_r[:], in_=me[:],
                                        axis=mybir.AxisListType.X, op=OP.add)
                sem_ps = psZp.tile([1, 1], dt.float32, tag="fin")
                nc.tensor.matmul(out=sem_ps[:], lhsT=ones32[:], rhs=me_r[:],
                                 start=True, stop=True)
                nc.vector.tensor_copy(out=out_sb[0:1, OC_EM:OC_EM + 1],
                                      in_=sem_ps[:])
                mt = miscp.tile([K, K], dt.float32, tag="mt")
                nc.vector.tensor_tensor(out=mt[:], in0=m_tr[:], in1=trans_sb,
                                        op=OP.mult)
                mt_r = miscp.tile([K, 1], dt.float32, tag="mt_r")
                nc.vector.tensor_reduce(out=mt_r[:], in_=mt[:],
                                        axis=mybir.AxisListType.X, op=OP.add)
                str_ps = psZp.tile([1, 1], dt.float32, tag="fin")
                nc.tensor.matmul(out=str_ps[:], lhsT=ones32[:], rhs=mt_r[:],
                                 start=True, stop=True)
                nc.vector.tensor_copy(out=out_sb[0:1, OC_TR:OC_TR + 1],
                                      in_=str_ps[:])

            # ---- head: unblock superstep 0 fast ----
            nc.scalar.activation(out=em0[:], in_=em01[:, 0:BG],
                                 func=AF.Exp, bias=expbias[:])
            nc.scalar.activation(out=em511[:], in_=em01[:, BG:2 * BG],
                                 func=AF.Exp, bias=expbias[:])
            exp_slots(0, 1)
            exp_slots(1, 7)
            exp_slots(8, 24)
            prep_oh(0)
            prep_oh(15)
            queue_mtr_chunk(15)
            nc.sync.dma_start(out=packf[:], in_=packf_d[:])
            # raw chunks for groups 0's golds were loaded at the head; the
            # remaining emt group blocks + raw chunks stream in on SP, one
            # group / chunk-pair ahead of use
            nc.sync.dma_start(out=emt[:, EMT_GRP:2 * EMT_GRP],
                              in_=emt_d[:, EMT_GRP:2 * EMT_GRP])
            raw_pairs = [((i, True), (15 - i, False))
                         for i in range(1, NCHUNKS // 2)]   # chunk pairs 1..7

            def dma_raw(c, fwd_side):
                rawp = rawFp if fwd_side else rawBp
                raw = rawp.tile([128, CHUNK_T * K], dt.bfloat16)
                nc.sync.dma_start(
                    out=raw[:], in_=em_d[:, c * CHUNK_T:(c + 1) * CHUNK_T, :])
                raw_tiles[c] = raw

            dma_raw(*raw_pairs[0][0])
            dma_raw(*raw_pairs[0][1])

            gold_done = [False]
            for h in range(N_GROUPS):
                # prefetch next group's emt block + raw chunk pair
                if h + 2 < N_GROUPS:
                    nc.sync.dma_start(
                        out=emt[:, (h + 2) * EMT_GRP:(h + 3) * EMT_GRP],
                        in_=emt_d[:, (h + 2) * EMT_GRP:(h + 3) * EMT_GRP])
                if h + 1 < len(raw_pairs) + 1 and h + 1 <= 6:
                    dma_raw(*raw_pairs[h + 1][0])
                    dma_raw(*raw_pairs[h + 1][1])
                # one-hots for this group's raw chunk pair
                if h < len(raw_pairs):
                    (cf, _), (cb, _) = raw_pairs[h]
                    prep_oh(cf)
                    prep_oh(cb)
                    queue_mtr_chunk(cf - 1)
                    queue_mtr_chunk(cb)
                    if h == 6:
                        queue_mtr_chunk(7)
                for s in range(h * SS_GROUP, (h + 1) * SS_GROUP):
                    cur_s[0] = s
                    # gold doses: released to the scheduler only around this
                    # superstep's simulated time (emitted before the chain
                    # matmuls so their stream slots precede them) — without
                    # the hold, the scheduler backfills dozens of ready gold
                    # matmuls into every chain-wait window and the lowered
                    # snapshot semaphores bill the chain for all of them
                    with tc.tile_wait_until(_EST(s)):
                        pop_gold(s)
                    superstep(s)
                    # next group's pairs exp, diced to 8-slot pieces emitted
                    # at distinct supersteps
                    if h + 1 < N_GROUPS and s % 8 == 4:
                        exp_slots((h + 1) * SS_GROUP + ((s // 8) % 4) * 8, 8)
            with tc.tile_wait_until(_EST(TAU)):
                while pending_gold:
                    pending_gold.pop(0)[1]()
            # finalize MUST be scheduled after every accumulating matmul:
            # Tile's dep tracker doesn't order PSUM-group readers behind all
            # group members (skip_group_check), so force it via release time
            # — its stream slot then snapshots semaphores past all of them
            with tc.tile_wait_until(_EST(TAU + 1)):
                emit_gold_finalize()

            # ---- stitch: Z_b = sum_i q_256[i,b] * rho_256[i,b] ----
            u = miscp.tile([128, BG], dt.float32, tag="u")
            nc.vector.tensor_tensor(out=u[:], in0=st["q_rhs"],
                                    in1=st["rho"][:, BG:2 * BG], op=OP.mult)
            z_ps = psNp.tile([G, BG], dt.float32, tag="s_ps")
            nc.tensor.matmul(out=z_ps[:], lhsT=onesbd_f, rhs=u[:],
                             start=True, stop=True)
            nc.vector.tensor_copy(out=out_sb[:, OC_Z:OC_Z + BG], in_=z_ps[:])

            nc.sync.dma_start(out=out_d[:], in_=out_sb[:])

    nc.compile()
    return nc


def _host_constants(transitions):
    """Host-prepared constant packs + the exact scale correction."""
    import ml_dtypes
    Tr64 = np.asarray(transitions, dtype=np.float64)
    expT = np.exp(Tr64)
    a = float(np.log(expT.sum() / K))
    Etil = (expT * math.exp(-a)).astype(np.float32)

    wf = np.kron(np.eye(G, dtype=np.float32), Etil)
    wb = np.kron(np.eye(G, dtype=np.float32), Etil.T.copy())
    onesbd = np.kron(np.eye(G, dtype=np.float32), np.ones((K, 1), np.float32))
    onesbc = np.kron(np.eye(G, dtype=np.float32), np.ones((1, K), np.float32))
    iota32 = np.tile(np.arange(K, dtype=np.float32), (128, 1))

    packbf = np.zeros((128, PBF_COLS), dtype=np.float32)
    packbf[:, PBF_WF:PBF_WF + 128] = wf
    packbf[:, PBF_WB:PBF_WB + 128] = wb
    packbf[:, PBF_OBD:PBF_OBD + G] = onesbd
    packbf[:, PBF_IOTA:PBF_IOTA + K] = iota32
    packbf = packbf.astype(ml_dtypes.bfloat16)

    packf = np.zeros((128, PF_COLS), dtype=np.float32)
    packf[:, PF_OBD:PF_OBD + G] = onesbd
    packf[0:G, PF_OBC:PF_OBC + 128] = onesbc
    packf[0:K, PF_EYE:PF_EYE + K] = np.eye(K, dtype=np.float32)
    packf[0:K, PF_TRANS:PF_TRANS + K] = np.asarray(transitions, np.float32)

    corr = (T - 1) * a + T * (-EXP_BIAS)
    return {"packbf": packbf, "packf": packf}, corr


def _host_emt(em_bf):
    """Slot-ordered pre-transposed emissions per core.

    em_bf: [B, T, K] bf16.  Returns (emt [B//BSH stacked 128, EMT_COLS],
    em01 [.., 2*BG]): partition p = 32*g + k, col-block s = 64 cols
    [fwd em_{s+1} (32 batch) | bwd em_{510-s}]; bwd half of slots 254/255
    is zero-filled (never read).
    """
    import ml_dtypes
    # [NCORES, G, BG, T, K] -> [NCORES, G, K, T, BG] (partition = g*K + k)
    X = em_bf.reshape(NCORES, G, BG, T, K).transpose(0, 1, 4, 3, 2)
    X = np.ascontiguousarray(X).reshape(NCORES, 128, T, BG)
    idx_f = np.arange(1, TAU + 1)               # t = 1..256
    idx_b = 510 - np.arange(TAU)                # t = 510..255
    fwd = X[:, :, idx_f, :]                     # [NC, 128, 256, BG]
    bwd = X[:, :, idx_b, :].copy()
    bwd[:, :, 254:, :] = 0
    emt = np.empty((NCORES, 128, TAU, 2 * BG), dtype=em_bf.dtype)
    emt[:, :, :, 0:BG] = fwd
    emt[:, :, :, BG:] = bwd
    emt = emt.reshape(NCORES, 128, EMT_COLS)
    em01 = np.concatenate([X[:, :, 0, :], X[:, :, T - 1, :]], axis=2)
    return emt, em01


def _host_reduce(outs, corr):
    """Combine per-core [G, OUT_COLS] outputs into the scalar loss."""
    total = 0.0
    for o in outs:
        o = np.asarray(o, dtype=np.float64).reshape(G, OUT_COLS)
        logz = np.log(o[:, OC_Z:OC_Z + BG])              # [G, BG]
        for r in range(N_RENORM):
            sps = o[:, OC_SPS + r * 2 * BG:OC_SPS + (r + 1) * 2 * BG]
            logz = logz + np.log(sps[:, 0:BG]) + np.log(sps[:, BG:2 * BG])
        total += logz.sum() - o[0, OC_EM] - o[0, OC_TR]
    return total / B + corr


def _host_onehot(tags):
    """Host-built one-hot [B, T*K] bf16: oh[b, t*K+j] = (tags[b,t] == j)."""
    import ml_dtypes
    tags = np.asarray(tags).astype(np.int32)
    oh = (tags[:, :, None] == np.arange(K, dtype=np.int32)[None, None, :])
    return np.ascontiguousarray(
        oh.astype(ml_dtypes.bfloat16).reshape(B, T * K))


def kernel(emissions, tags, mask, transitions):
    import ml_dtypes
    from concourse.bass_utils import run_bass_kernel_spmd

    em_bf = np.ascontiguousarray(
        np.asarray(emissions, dtype=np.float32).astype(ml_dtypes.bfloat16))
    oh = _host_onehot(tags)
    transitions = np.ascontiguousarray(np.asarray(transitions, dtype=np.float32))

    if "nc" not in _PROGRAM_CACHE:
        _PROGRAM_CACHE["nc"] = _build_program()
    nc = _PROGRAM_CACHE["nc"]

    consts, corr = _host_constants(transitions)
    emt, em01 = _host_emt(em_bf)
    core_ids = list(range(NCORES))
    in_maps = []
    for c in core_ids:
        sl = slice(c * BSH, (c + 1) * BSH)
        m = {"emissions": em_bf[sl], "oh": oh[sl],
             "emt": emt[c], "em01": em01[c]}
        m.update(consts)
        in_maps.append(m)

    res = run_bass_kernel_spmd(nc, in_maps, core_ids)
    _PROGRAM_CACHE["last_results"] = res
    loss = _host_reduce([r["out"] for r in res.results], corr)
    return np.float32(loss)


# revision 54
# speedup vs baseline: 11.3078x; 11.3078x over previous
"""Trainium2 Bass kernel for the CRF loss (nn_CRFLayer).

Full-input contract: kernel(**inputs) takes the full [1024,512,32] emissions,
[1024,512] tags, [1024,512] mask (all-ones by construction), [32,32]
transitions; returns the scalar f32 loss.

Strategy (8 NeuronCores, data-parallel over batch, 128 rows/core):
  - Exp-space forward algorithm:  q_t = (E~^T q_{t-1}) * exp(em_t - 1/2),
    with E~ = exp(transitions) * exp(-a) (a: global scale fold, corrected
    exactly on the host afterwards).
  - Bidirectional split: a forward chain covers t=1..256, an independent
    backward chain covers t=511..257; they stitch at t=256 via
    Z_b = sum_i q_256[i,b] * rho_256[i,b], halving the serial depth.
  - Layout: state [128 partitions = (4 batch-groups x 32 tags), 32 free =
    batch-in-group].  The K=32 contraction is one 128x128 block-diagonal
    bf16 matmul (kron(I4, E~)) per direction, both into one PSUM bank.
  - The emissions arrive TWICE from the host, bf16: raw [b,(t,k)] layout
    (feeds the gold-score matmuls) and pre-transposed slot-ordered layout
    (feeds the chain), so no on-device transposes exist at all.  ACT
    exponentiates slot blocks into the "pairs" buffer one superstep-group
    ahead of use; a single DVE multiply per superstep advances both
    chains.  DVE runs nothing else (the critical PE->DVE->PE loop stays
    clean: Tile lowers semaphore waits as stream-position snapshots, so
    any engine work scheduled into a chain wait window would add its own
    execution time to the chain).
  - Each multiply writes a dedicated SBUF buffer (no pool recycling), so
    its single hardware wait slot carries the PE semaphore inline instead
    of an extra EventSemaphore instruction.
  - Renormalization (every 64 steps) runs OFF the critical chain: the
    per-batch reciprocal-of-sums is multiplied into the pairs buffer 12
    supersteps ahead (linearity makes deferred rescaling exact); the raw
    sums ship to the host, which applies exact log corrections.
  - Gold path score via host-built one-hot pieces OH[b,t,j] =
    (tags[b,t]==j), streamed in by DMA:
      * emission score  matrix M_em = sum_{b,t} em x OH
      * transition score matrix M_tr = sum OH_t x OH_{t+1}
    both as long PSUM-accumulating PE matmul chains, released ~5 matmuls
    per superstep via tile_wait_until so the scheduler cannot backfill
    them into chain wait windows (the lowered snapshot semaphores would
    bill the chain for them).  The matrices ship raw; the host takes
    trace(M_em) and sum(M_tr * transitions).
  - Per-core output [32, 290] f32: per-batch stitch sums Z, the 3 renorm
    sum blocks, and the two gold matrices; host does the logs, the gold
    contractions, the subtraction, the scale correction, and the mean.
"""

import math
import numpy as np

B, T, K = 1024, 512, 32
NCORES = 8
BSH = B // NCORES          # 128 batch rows per core
G = 4                      # batch groups stacked on partitions
BG = BSH // G              # 32 batch per group
TAU = 256                  # stitch point
NORM_EVERY = 64
N_RENORM = TAU // NORM_EVERY - 1   # 3
CHUNK_T = 32               # timesteps per raw chunk ([128, 1024])
NCHUNKS = T // CHUNK_T     # 16
SS_GROUP = 32              # supersteps per group
N_GROUPS = TAU // SS_GROUP # 8
EXP_BIAS = -0.5            # em~ = exp(em + EXP_BIAS)
GOLD_DOSE = 5              # gold-score matmuls emitted per superstep
OH_DELAY = 6               # supersteps between one-hot emission and its golds

# const pack layouts
PBF_WF, PBF_WB, PBF_OBD, PBF_IOTA = 0, 128, 256, 260
PBF_COLS = 292
PF_OBD, PF_OBC, PF_EYE, PF_TRANS = 0, 4, 132, 164
PF_COLS = 196
# output layout: [K, OUT_COLS]; z/renorm blocks live in rows 0:G, the two
# gold reduction vectors occupy one column each across all K rows
OC_Z, OC_EM, OC_TR, OC_SPS = 0, 32, 33, 34
OC_MEM = 34 + N_RENORM * 2 * BG         # 226: m_em matrix block [K cols]
OC_MTR = OC_MEM + K                     # 258: m_tr matrix block [K cols]
OUT_COLS = OC_MTR + K                   # 290

EMT_COLS = TAU * 2 * BG    # slot-ordered emissions, [128, 16384]
EM01_COLS = 2 * BG         # [em0 | em511]

# scheduler release-time estimate for superstep s (ms for tile_wait_until)
HEAD_NS = 5000
PERIOD_NS = 600


def _EST(s):
    return (HEAD_NS + PERIOD_NS * s) / 1e6

_PROGRAM_CACHE = {}


def _build_program():
    """Builds the single-core SPMD bass program."""
    import concourse.bass as bass
    import concourse.mybir as mybir
    import concourse.bacc as bacc
    from concourse import tile
    from concourse.bass_types import AP

    dt = mybir.dt
    AF = mybir.ActivationFunctionType
    OP = mybir.AluOpType

    nc = bacc.Bacc("TRN2", target_bir_lowering=False, debug=False)

    em_d = nc.declare_dram_parameter("emissions", [BSH, T, K], dt.bfloat16, isOutput=False)
    emt_d = nc.declare_dram_parameter("emt", [128, EMT_COLS], dt.bfloat16, isOutput=False)
    em01_d = nc.declare_dram_parameter("em01", [128, EM01_COLS], dt.bfloat16, isOutput=False)
    oh_d = nc.declare_dram_parameter("oh", [BSH, T * K], dt.bfloat16, isOutput=False)
    packbf_d = nc.declare_dram_parameter("packbf", [128, PBF_COLS], dt.bfloat16, isOutput=False)
    packf_d = nc.declare_dram_parameter("packf", [128, PF_COLS], dt.float32, isOutput=False)
    out_d = nc.declare_dram_parameter("out", [K, OUT_COLS], dt.float32, isOutput=True)

    with tile.TileContext(nc) as tc:
        with (
            tc.tile_pool(name="const", bufs=1) as constp,
            tc.tile_pool(name="rawF", bufs=4) as rawFp,
            tc.tile_pool(name="rawB", bufs=4) as rawBp,
            tc.tile_pool(name="misc", bufs=2) as miscp,
            tc.tile_pool(name="psP", bufs=3, space="PSUM") as psPp,
            tc.tile_pool(name="psN", bufs=1, space="PSUM") as psNp,
            tc.tile_pool(name="psZ", bufs=1, space="PSUM") as psZp,
            tc.tile_pool(name="psME", bufs=1, space="PSUM") as psMEp,
            tc.tile_pool(name="psMT", bufs=1, space="PSUM") as psMTp,
        ):
            # ---- head-critical DMAs, all on SP's (fast) queue in
            # criticality order; ACT's sequencer does only exps ----
            EMT_GRP = SS_GROUP * 2 * BG     # 2048 cols per superstep-group
            EMT_HEAD = 8 * 2 * BG           # first 8 slots land first
            em01 = constp.tile([128, EM01_COLS], dt.bfloat16)
            nc.sync.dma_start(out=em01[:], in_=em01_d[:])
            emt = constp.tile([128, EMT_COLS], dt.bfloat16)
            nc.sync.dma_start(out=emt[:, 0:EMT_HEAD], in_=emt_d[:, 0:EMT_HEAD])
            packbf = constp.tile([128, PBF_COLS], dt.bfloat16)
            nc.sync.dma_start(out=packbf[:], in_=packbf_d[:])
            nc.sync.dma_start(out=emt[:, EMT_HEAD:EMT_GRP],
                              in_=emt_d[:, EMT_HEAD:EMT_GRP])

            raw0 = rawFp.tile([128, CHUNK_T * K], dt.bfloat16)
            nc.sync.dma_start(out=raw0[:], in_=em_d[:, 0:CHUNK_T, :])
            raw15 = rawBp.tile([128, CHUNK_T * K], dt.bfloat16)
            nc.sync.dma_start(
                out=raw15[:], in_=em_d[:, (NCHUNKS - 1) * CHUNK_T:, :])

            packf = constp.tile([128, PF_COLS], dt.float32)
            oh_sb = constp.tile([BSH, T * K], dt.bfloat16)

            wf = packbf[:, PBF_WF:PBF_WF + 128]
            wb = packbf[:, PBF_WB:PBF_WB + 128]
            onesbd_bf = packbf[:, PBF_OBD:PBF_OBD + G]
            onesbd_f = packf[:, PF_OBD:PF_OBD + G]
            onesbc_f = packf[0:G, PF_OBC:PF_OBC + 128]

            expbias = constp.tile([128, 1], dt.float32)
            nc.vector.memset(expbias[:], EXP_BIAS)
            # dummy activation: hoists the 1.3us ACT table load to t~=1us,
            # off the head-critical path (it would otherwise run right
            # before the first real exp, after its input DMA lands)
            atl = constp.tile([1, 1], dt.float32)
            nc.scalar.activation(out=atl[:], in_=expbias[0:1, :],
                                 func=AF.Exp)

            out_sb = constp.tile([K, OUT_COLS], dt.float32)
            nc.vector.memset(out_sb[:], 0.0)

            # em~ pairs buffer: slot s = [fwd em~_{s+1} | bwd em~_{510-s}]
            pairs = constp.tile([128, TAU * 2 * BG], dt.bfloat16)
            em0 = constp.tile([128, BG], dt.bfloat16)
            em511 = constp.tile([128, BG], dt.bfloat16)

            oh_pieces = [None] * NCHUNKS
            raw_tiles = {0: raw0, 15: raw15}
            m_em = psMEp.tile([K, K], dt.float32, tag="m_em")
            m_tr = psMTp.tile([K, K], dt.float32, tag="m_tr")

            # ---- gold-score machinery (lazy, dosed) ----
            pending_gold = []   # heap-ish FIFO of (min_superstep, closure)
            mem_count = [0]
            mtr_count = [0]
            N_MEM = T
            N_MTR = T - 1
            cur_s = [0]

            def gold_mem(c, k):
                def emit():
                    nc.tensor.matmul(
                        out=m_em[:],
                        lhsT=raw_tiles[c][:, k * K:(k + 1) * K],
                        rhs=oh_pieces[c][:, k * K:(k + 1) * K],
                        start=(mem_count[0] == 0),
                        stop=(mem_count[0] == N_MEM - 1),
                        skip_group_check=True)
                    mem_count[0] += 1
                return emit

            def gold_mtr(t):
                def emit():
                    ca, sa = t // CHUNK_T, t % CHUNK_T
                    cb, sb = (t + 1) // CHUNK_T, (t + 1) % CHUNK_T
                    nc.tensor.matmul(
                        out=m_tr[:],
                        lhsT=oh_pieces[ca][:, sa * K:(sa + 1) * K],
                        rhs=oh_pieces[cb][:, sb * K:(sb + 1) * K],
                        start=(mtr_count[0] == 0),
                        stop=(mtr_count[0] == N_MTR - 1),
                        skip_group_check=True)
                    mtr_count[0] += 1
                return emit

            oh_emit_s = {}

            def prep_oh(c):
                # one-hots are host-built and DMA'd (no legal on-device
                # engine computes is_equal without touching DVE/ACT, whose
                # snapshot semaphores would couple it to the chain)
                nc.sync.dma_start(
                    out=oh_sb[:, c * CHUNK_T * K:(c + 1) * CHUNK_T * K],
                    in_=oh_d[:, c * CHUNK_T * K:(c + 1) * CHUNK_T * K])
                oh_pieces[c] = oh_sb[:, c * CHUNK_T * K:(c + 1) * CHUNK_T * K]
                oh_emit_s[c] = cur_s[0]
                ready = cur_s[0] + OH_DELAY
                for k in range(CHUNK_T):
                    pending_gold.append((ready, gold_mem(c, k)))

            def queue_mtr_chunk(c):
                # both one-hots of every t in this range exist already
                ready = max(oh_emit_s.get(c, 0),
                            oh_emit_s.get(c + 1, 0)) + OH_DELAY
                lo = c * CHUNK_T
                hi = min((c + 1) * CHUNK_T, T - 1)
                for t in range(lo, hi):
                    pending_gold.append((ready, gold_mtr(t)))

            def pop_gold(s):
                n = 0
                while n < GOLD_DOSE and pending_gold and pending_gold[0][0] <= s:
                    pending_gold.pop(0)[1]()
                    n += 1

            # ---- pairs exp: slot blocks from the pre-transposed emt ----
            def exp_slots(lo, n):
                nc.scalar.activation(
                    out=pairs[:, lo * 2 * BG:(lo + n) * 2 * BG],
                    in_=emt[:, lo * 2 * BG:(lo + n) * 2 * BG],
                    func=AF.Exp, bias=expbias[:])

            # ---- chain state ----
            st = {"q_rhs": em0[:], "v_rhs": em511[:], "rho": None}
            renorm_idx = [0]

            def superstep(s):
                ps = psPp.tile([128, 2 * BG], dt.float32, tag="qv")
                nc.tensor.matmul(out=ps[:, 0:BG], lhsT=wf, rhs=st["q_rhs"],
                                 start=True, stop=True)
                if s <= 254:
                    nc.tensor.matmul(out=ps[:, BG:2 * BG], lhsT=wb,
                                     rhs=st["v_rhs"], start=True, stop=True)
                if s <= 253:
                    # dedicated buffer per superstep: no pool WAW self-wait,
                    # so the mult's single wait slot holds the PE sem inline
                    qv = constp.tile([128, 2 * BG], dt.bfloat16, tag=f"qv{s}")
                    nc.vector.tensor_tensor(
                        out=qv[:], in0=ps[:],
                        in1=pairs[:, 2 * BG * s:2 * BG * (s + 1)], op=OP.mult)
                    st["q_rhs"] = qv[:, 0:BG]
                    st["v_rhs"] = qv[:, BG:2 * BG]
                    if (s + 1) % NORM_EVERY == 0 and s + 1 < TAU:
                        # off-chain renorm: rescale pairs slot s+12 by
                        # 1/sums; raw sums ship to host for exact log fixup.
                        # Each sub-op is released into a distinct later
                        # window so no chain wait snapshots a long residue.
                        r = renorm_idx[0]
                        renorm_idx[0] += 1
                        s_ps = psNp.tile([G, 2 * BG], dt.float32, tag="s_ps")
                        nc.tensor.matmul(out=s_ps[:], lhsT=onesbd_bf,
                                         rhs=qv[:], start=True, stop=True)
                        nc.vector.tensor_copy(
                            out=out_sb[0:G, OC_SPS + r * 2 * BG:
                                       OC_SPS + (r + 1) * 2 * BG],
                            in_=s_ps[:])
                        rs = miscp.tile([G, 2 * BG], dt.float32, tag="rs")
                        nc.vector.reciprocal(out=rs[:], in_=s_ps[:])
                        bc_ps = psNp.tile([128, 2 * BG], dt.float32, tag="bc_ps")
                        nc.tensor.matmul(out=bc_ps[:], lhsT=onesbc_f,
                                         rhs=rs[:], start=True, stop=True)
                        sl = pairs[:, 2 * BG * (s + 12):2 * BG * (s + 13)]
                        nc.vector.tensor_tensor(out=sl, in0=sl, in1=bc_ps[:],
                                                op=OP.mult)
                else:
                    qv = constp.tile([128, BG], dt.bfloat16, tag=f"qv{s}")
                    nc.vector.tensor_tensor(
                        out=qv[:], in0=ps[:, 0:BG],
                        in1=pairs[:, 2 * BG * s:2 * BG * s + BG], op=OP.mult)
                    st["q_rhs"] = qv[:]
                    if s == 254:
                        st["rho"] = ps

            def emit_gold_finalize():
                # ship the raw m_em / m_tr matrices; the host computes
                # trace(m_em) and sum(m_tr * transitions)
                nc.vector.tensor_copy(out=out_sb[0:K, OC_MEM:OC_MEM + K],
                                      in_=m_em[:])
                nc.vector.tensor_copy(out=out_sb[0:K, OC_MTR:OC_MTR + K],
                                      in_=m_tr[:])

            # ---- head: unblock superstep 0 fast ----
            nc.scalar.activation(out=em0[:], in_=em01[:, 0:BG],
                                 func=AF.Exp, bias=expbias[:])
            nc.scalar.activation(out=em511[:], in_=em01[:, BG:2 * BG],
                                 func=AF.Exp, bias=expbias[:])
            exp_slots(0, 1)
            exp_slots(1, 7)
            exp_slots(8, 24)
            prep_oh(0)
            prep_oh(15)
            queue_mtr_chunk(15)
            nc.sync.dma_start(out=packf[:], in_=packf_d[:])
            # raw chunks for groups 0's golds were loaded at the head; the
            # remaining emt group blocks + raw chunks stream in on SP, one
            # group / chunk-pair ahead of use
            nc.sync.dma_start(out=emt[:, EMT_GRP:2 * EMT_GRP],
                              in_=emt_d[:, EMT_GRP:2 * EMT_GRP])
            raw_pairs = [((i, True), (15 - i, False))
                         for i in range(1, NCHUNKS // 2)]   # chunk pairs 1..7

            def dma_raw(c, fwd_side):
                rawp = rawFp if fwd_side else rawBp
                raw = rawp.tile([128, CHUNK_T * K], dt.bfloat16)
                nc.sync.dma_start(
                    out=raw[:], in_=em_d[:, c * CHUNK_T:(c + 1) * CHUNK_T, :])
                raw_tiles[c] = raw

            dma_raw(*raw_pairs[0][0])
            dma_raw(*raw_pairs[0][1])

            gold_done = [False]
            for h in range(N_GROUPS):
                # prefetch next group's emt block + raw chunk pair
                if h + 2 < N_GROUPS:
                    nc.sync.dma_start(
                        out=emt[:, (h + 2) * EMT_GRP:(h + 3) * EMT_GRP],
                        in_=emt_d[:, (h + 2) * EMT_GRP:(h + 3) * EMT_GRP])
                if h + 1 < len(raw_pairs) + 1 and h + 1 <= 6:
                    dma_raw(*raw_pairs[h + 1][0])
                    dma_raw(*raw_pairs[h + 1][1])
                # one-hots for this group's raw chunk pair
                if h < len(raw_pairs):
                    (cf, _), (cb, _) = raw_pairs[h]
                    prep_oh(cf)
                    prep_oh(cb)
                    queue_mtr_chunk(cf - 1)
                    queue_mtr_chunk(cb)
                    if h == 6:
                        queue_mtr_chunk(7)
                for s in range(h * SS_GROUP, (h + 1) * SS_GROUP):
                    cur_s[0] = s
                    # gold doses: released to the scheduler only around this
                    # superstep's simulated time (emitted before the chain
                    # matmuls so their stream slots precede them) — without
                    # the hold, the scheduler backfills dozens of ready gold
                    # matmuls into every chain-wait window and the lowered
                    # snapshot semaphores bill the chain for all of them
                    with tc.tile_wait_until(_EST(s)):
                        pop_gold(s)
                    superstep(s)
                    # next group's pairs exp, diced to 8-slot pieces emitted
                    # at distinct supersteps
                    if h + 1 < N_GROUPS and s % 8 == 4:
                        exp_slots((h + 1) * SS_GROUP + ((s // 8) % 4) * 8, 8)
            with tc.tile_wait_until(_EST(TAU)):
                while pending_gold:
                    pending_gold.pop(0)[1]()
            # finalize MUST be scheduled after every accumulating matmul:
            # Tile's dep tracker doesn't order PSUM-group readers behind all
            # group members (skip_group_check), so force it via release time
            # — its stream slot then snapshots semaphores past all of them
            with tc.tile_wait_until(_EST(230)):
                emit_gold_finalize()

            # ---- stitch: Z_b = sum_i q_256[i,b] * rho_256[i,b] ----
            u = miscp.tile([128, BG], dt.float32, tag="u")
            nc.vector.tensor_tensor(out=u[:], in0=st["q_rhs"],
                                    in1=st["rho"][:, BG:2 * BG], op=OP.mult)
            z_ps = psNp.tile([G, BG], dt.float32, tag="s_ps")
            nc.tensor.matmul(out=z_ps[:], lhsT=onesbd_f, rhs=u[:],
                             start=True, stop=True)
            nc.vector.tensor_copy(out=out_sb[0:G, OC_Z:OC_Z + BG], in_=z_ps[:])

            nc.sync.dma_start(out=out_d[:], in_=out_sb[:])

    nc.compile()
    return nc


def _host_constants(transitions):
    """Host-prepared constant packs + the exact scale correction."""
    import ml_dtypes
    Tr64 = np.asarray(transitions, dtype=np.float64)
    expT = np.exp(Tr64)
    a = float(np.log(expT.sum() / K))
    Etil = (expT * math.exp(-a)).astype(np.float32)

    wf = np.kron(np.eye(G, dtype=np.float32), Etil)
    wb = np.kron(np.eye(G, dtype=np.float32), Etil.T.copy())
    onesbd = np.kron(np.eye(G, dtype=np.float32), np.ones((K, 1), np.float32))
    onesbc = np.kron(np.eye(G, dtype=np.float32), np.ones((1, K), np.float32))
    iota32 = np.tile(np.arange(K, dtype=np.float32), (128, 1))

    packbf = np.zeros((128, PBF_COLS), dtype=np.float32)
    packbf[:, PBF_WF:PBF_WF + 128] = wf
    packbf[:, PBF_WB:PBF_WB + 128] = wb
    packbf[:, PBF_OBD:PBF_OBD + G] = onesbd
    packbf[:, PBF_IOTA:PBF_IOTA + K] = iota32
    packbf = packbf.astype(ml_dtypes.bfloat16)

    packf = np.zeros((128, PF_COLS), dtype=np.float32)
    packf[:, PF_OBD:PF_OBD + G] = onesbd
    packf[0:G, PF_OBC:PF_OBC + 128] = onesbc
    packf[0:K, PF_EYE:PF_EYE + K] = np.eye(K, dtype=np.float32)
    packf[0:K, PF_TRANS:PF_TRANS + K] = np.asarray(transitions, np.float32)

    corr = (T - 1) * a + T * (-EXP_BIAS)
    return {"packbf": packbf, "packf": packf}, corr


def _host_emt(em_bf):
    """Slot-ordered pre-transposed emissions per core.

    em_bf: [B, T, K] bf16.  Returns (emt [B//BSH stacked 128, EMT_COLS],
    em01 [.., 2*BG]): partition p = 32*g + k, col-block s = 64 cols
    [fwd em_{s+1} (32 batch) | bwd em_{510-s}]; bwd half of slots 254/255
    is zero-filled (never read).
    """
    import ml_dtypes
    # [NCORES, G, BG, T, K] -> [NCORES, G, K, T, BG] (partition = g*K + k)
    X = em_bf.reshape(NCORES, G, BG, T, K).transpose(0, 1, 4, 3, 2)
    X = np.ascontiguousarray(X).reshape(NCORES, 128, T, BG)
    idx_f = np.arange(1, TAU + 1)               # t = 1..256
    idx_b = 510 - np.arange(TAU)                # t = 510..255
    fwd = X[:, :, idx_f, :]                     # [NC, 128, 256, BG]
    bwd = X[:, :, idx_b, :].copy()
    bwd[:, :, 254:, :] = 0
    emt = np.empty((NCORES, 128, TAU, 2 * BG), dtype=em_bf.dtype)
    emt[:, :, :, 0:BG] = fwd
    emt[:, :, :, BG:] = bwd
    emt = emt.reshape(NCORES, 128, EMT_COLS)
    em01 = np.concatenate([X[:, :, 0, :], X[:, :, T - 1, :]], axis=2)
    return emt, em01


def _host_reduce(outs, corr, trans):
    """Combine per-core [K, OUT_COLS] outputs into the scalar loss."""
    total = 0.0
    for o in outs:
        o = np.asarray(o, dtype=np.float64).reshape(K, OUT_COLS)
        logz = np.log(o[0:G, OC_Z:OC_Z + BG])            # [G, BG]
        for r in range(N_RENORM):
            sps = o[0:G, OC_SPS + r * 2 * BG:OC_SPS + (r + 1) * 2 * BG]
            logz = logz + np.log(sps[:, 0:BG]) + np.log(sps[:, BG:2 * BG])
        m_em = o[0:K, OC_MEM:OC_MEM + K]
        m_tr = o[0:K, OC_MTR:OC_MTR + K]
        total += logz.sum() - np.trace(m_em) - (m_tr * trans).sum()
    return total / B + corr


def _host_onehot(tags):
    """Host-built one-hot [B, T*K] bf16: oh[b, t*K+j] = (tags[b,t] == j)."""
    import ml_dtypes
    tags = np.asarray(tags).astype(np.int32)
    oh = (tags[:, :, None] == np.arange(K, dtype=np.int32)[None, None, :])
    return np.ascontiguousarray(
        oh.astype(ml_dtypes.bfloat16).reshape(B, T * K))


def kernel(emissions, tags, mask, transitions):
    import ml_dtypes
    from concourse.bass_utils import run_bass_kernel_spmd

    em_bf = np.ascontiguousarray(
        np.asarray(emissions, dtype=np.float32).astype(ml_dtypes.bfloat16))
    oh = _host_onehot(tags)
    transitions = np.ascontiguousarray(np.asarray(transitions, dtype=np.float32))

    if "nc" not in _PROGRAM_CACHE:
        _PROGRAM_CACHE["nc"] = _build_program()
    nc = _PROGRAM_CACHE["nc"]

    consts, corr = _host_constants(transitions)
    emt, em01 = _host_emt(em_bf)
    core_ids = list(range(NCORES))
    in_maps = []
    for c in core_ids:
        sl = slice(c * BSH, (c + 1) * BSH)
        m = {"emissions": em_bf[sl], "oh": oh[sl],
             "emt": emt[c], "em01": em01[c]}
        m.update(consts)
        in_maps.append(m)

    res = run_bass_kernel_spmd(nc, in_maps, core_ids)
    _PROGRAM_CACHE["last_results"] = res
    loss = _host_reduce([r["out"] for r in res.results], corr,
                        np.asarray(transitions, np.float64))
    return np.float32(loss)


# revision 60
# speedup vs baseline: 11.4120x; 1.0092x over previous
"""Trainium2 Bass kernel for the CRF loss (nn_CRFLayer).

Full-input contract: kernel(**inputs) takes the full [1024,512,32] emissions,
[1024,512] tags, [1024,512] mask (all-ones by construction), [32,32]
transitions; returns the scalar f32 loss.

Strategy (8 NeuronCores, data-parallel over batch, 128 rows/core):
  - Exp-space forward algorithm:  q_t = (E~^T q_{t-1}) * exp(em_t - 1/2),
    with E~ = exp(transitions) * exp(-a) (a: global scale fold, corrected
    exactly on the host afterwards).
  - Bidirectional split: a forward chain covers t=1..256, an independent
    backward chain covers t=511..257; they stitch at t=256 via
    Z_b = sum_i q_256[i,b] * rho_256[i,b], halving the serial depth.
  - Layout: state [128 partitions = (4 batch-groups x 32 tags), 32 free =
    batch-in-group].  The K=32 contraction is one 128x128 block-diagonal
    bf16 matmul (kron(I4, E~)) per direction, both into one PSUM bank.
  - The emissions arrive TWICE from the host, bf16: raw [b,(t,k)] layout
    (feeds the gold-score matmuls) and pre-transposed slot-ordered layout
    (feeds the chain), so no on-device transposes exist at all.  ACT
    exponentiates slot blocks into the "pairs" buffer one superstep-group
    ahead of use; a single DVE multiply per superstep advances both
    chains.  DVE runs nothing else (the critical PE->DVE->PE loop stays
    clean: Tile lowers semaphore waits as stream-position snapshots, so
    any engine work scheduled into a chain wait window would add its own
    execution time to the chain).
  - Each multiply writes a dedicated SBUF buffer (no pool recycling), so
    its single hardware wait slot carries the PE semaphore inline instead
    of an extra EventSemaphore instruction.
  - Renormalization (every 64 steps) runs OFF the critical chain: the
    per-batch reciprocal-of-sums is multiplied into the pairs buffer 12
    supersteps ahead (linearity makes deferred rescaling exact); the raw
    sums ship to the host, which applies exact log corrections.
  - Gold path score via host-built one-hot pieces OH[b,t,j] =
    (tags[b,t]==j), streamed in by DMA:
      * emission score  matrix M_em = sum_{b,t} em x OH
      * transition score matrix M_tr = sum OH_t x OH_{t+1}
    both as long PSUM-accumulating PE matmul chains, released ~10 matmuls
    per superstep via tile_wait_until so the scheduler cannot backfill
    them all into one chain wait window (the lowered snapshot semaphores
    would bill the chain for them).  The matrices ship raw; the host
    takes trace(M_em) and sum(M_tr * transitions).
  - Per-core output [32, 290] f32: per-batch stitch sums Z, the 3 renorm
    sum blocks, and the two gold matrices; host does the logs, the gold
    contractions, the subtraction, the scale correction, and the mean.
"""

import math
import numpy as np

B, T, K = 1024, 512, 32
NCORES = 8
BSH = B // NCORES          # 128 batch rows per core
G = 4                      # batch groups stacked on partitions
BG = BSH // G              # 32 batch per group
TAU = 256                  # stitch point
NORM_EVERY = 64
N_RENORM = TAU // NORM_EVERY - 1   # 3
CHUNK_T = 32               # timesteps per raw chunk ([128, 1024])
NCHUNKS = T // CHUNK_T     # 16
SS_GROUP = 32              # supersteps per group
N_GROUPS = TAU // SS_GROUP # 8
EXP_BIAS = -0.5            # em~ = exp(em + EXP_BIAS)
GOLD_DOSE = 10             # gold-score matmuls emitted per superstep
OH_DELAY = 6               # supersteps between one-hot emission and its golds

# const pack layouts
PBF_WF, PBF_WB, PBF_OBD, PBF_IOTA = 0, 128, 256, 260
PBF_COLS = 292
PF_OBD, PF_OBC, PF_EYE, PF_TRANS = 0, 4, 132, 164
PF_COLS = 196
# output layout: [K, OUT_COLS]; z/renorm blocks live in rows 0:G, the two
# gold reduction vectors occupy one column each across all K rows
OC_Z, OC_EM, OC_TR, OC_SPS = 0, 32, 33, 34
OC_MEM = 34 + N_RENORM * 2 * BG         # 226: m_em matrix block [K cols]
OC_MTR = OC_MEM + K                     # 258: m_tr matrix block [K cols]
OUT_COLS = OC_MTR + K                   # 290

EMT_COLS = TAU * 2 * BG    # slot-ordered emissions, [128, 16384]
EM01_COLS = 2 * BG         # [em0 | em511]

# scheduler release-time estimate for superstep s (ms for tile_wait_until)
HEAD_NS = 5000
PERIOD_NS = 600


def _EST(s):
    return (HEAD_NS + PERIOD_NS * s) / 1e6

_PROGRAM_CACHE = {}


def _build_program():
    """Builds the single-core SPMD bass program."""
    import concourse.bass as bass
    import concourse.mybir as mybir
    import concourse.bacc as bacc
    from concourse import tile
    from concourse.bass_types import AP

    dt = mybir.dt
    AF = mybir.ActivationFunctionType
    OP = mybir.AluOpType

    nc = bacc.Bacc("TRN2", target_bir_lowering=False, debug=False)

    em_d = nc.declare_dram_parameter("emissions", [BSH, T, K], dt.bfloat16, isOutput=False)
    emt_d = nc.declare_dram_parameter("emt", [128, EMT_COLS], dt.bfloat16, isOutput=False)
    em01_d = nc.declare_dram_parameter("em01", [128, EM01_COLS], dt.bfloat16, isOutput=False)
    oh_d = nc.declare_dram_parameter("oh", [BSH, T * K], dt.bfloat16, isOutput=False)
    packbf_d = nc.declare_dram_parameter("packbf", [128, PBF_COLS], dt.bfloat16, isOutput=False)
    packf_d = nc.declare_dram_parameter("packf", [128, PF_COLS], dt.float32, isOutput=False)
    out_d = nc.declare_dram_parameter("out", [K, OUT_COLS], dt.float32, isOutput=True)

    with tile.TileContext(nc) as tc:
        with (
            tc.tile_pool(name="const", bufs=1) as constp,
            tc.tile_pool(name="rawF", bufs=4) as rawFp,
            tc.tile_pool(name="rawB", bufs=4) as rawBp,
            tc.tile_pool(name="misc", bufs=2) as miscp,
            tc.tile_pool(name="psP", bufs=3, space="PSUM") as psPp,
            tc.tile_pool(name="psN", bufs=1, space="PSUM") as psNp,
            tc.tile_pool(name="psZ", bufs=1, space="PSUM") as psZp,
            tc.tile_pool(name="psME", bufs=1, space="PSUM") as psMEp,
            tc.tile_pool(name="psMT", bufs=1, space="PSUM") as psMTp,
        ):
            # ---- head-critical DMAs, all on SP's (fast) queue in
            # criticality order; ACT's sequencer does only exps ----
            EMT_GRP = SS_GROUP * 2 * BG     # 2048 cols per superstep-group
            EMT_HEAD = 8 * 2 * BG           # first 8 slots land first
            em01 = constp.tile([128, EM01_COLS], dt.bfloat16)
            nc.sync.dma_start(out=em01[:], in_=em01_d[:])
            emt = constp.tile([128, EMT_COLS], dt.bfloat16)
            nc.sync.dma_start(out=emt[:, 0:EMT_HEAD], in_=emt_d[:, 0:EMT_HEAD])
            packbf = constp.tile([128, PBF_COLS], dt.bfloat16)
            nc.sync.dma_start(out=packbf[:], in_=packbf_d[:])
            nc.sync.dma_start(out=emt[:, EMT_HEAD:EMT_GRP],
                              in_=emt_d[:, EMT_HEAD:EMT_GRP])

            raw0 = rawFp.tile([128, CHUNK_T * K], dt.bfloat16)
            nc.sync.dma_start(out=raw0[:], in_=em_d[:, 0:CHUNK_T, :])
            raw15 = rawBp.tile([128, CHUNK_T * K], dt.bfloat16)
            nc.sync.dma_start(
                out=raw15[:], in_=em_d[:, (NCHUNKS - 1) * CHUNK_T:, :])

            packf = constp.tile([128, PF_COLS], dt.float32)

            wf = packbf[:, PBF_WF:PBF_WF + 128]
            wb = packbf[:, PBF_WB:PBF_WB + 128]
            onesbd_bf = packbf[:, PBF_OBD:PBF_OBD + G]
            onesbd_f = packf[:, PF_OBD:PF_OBD + G]
            onesbc_f = packf[0:G, PF_OBC:PF_OBC + 128]

            expbias = constp.tile([128, 1], dt.float32)
            nc.vector.memset(expbias[:], EXP_BIAS)
            # dummy activation: hoists the 1.3us ACT table load to t~=1us,
            # off the head-critical path (it would otherwise run right
            # before the first real exp, after its input DMA lands)
            atl = constp.tile([1, 1], dt.float32)
            nc.scalar.activation(out=atl[:], in_=expbias[0:1, :],
                                 func=AF.Exp)

            out_sb = constp.tile([K, OUT_COLS], dt.float32)
            nc.vector.memset(out_sb[:], 0.0)

            # em~ pairs buffer: slot s = [fwd em~_{s+1} | bwd em~_{510-s}]
            pairs = constp.tile([128, TAU * 2 * BG], dt.bfloat16)
            em0 = constp.tile([128, BG], dt.bfloat16)
            em511 = constp.tile([128, BG], dt.bfloat16)

            oh_pieces = [None] * NCHUNKS
            raw_tiles = {0: raw0, 15: raw15}
            m_em = psMEp.tile([K, K], dt.float32, tag="m_em")
            m_tr = psMTp.tile([K, K], dt.float32, tag="m_tr")

            # ---- gold-score machinery (lazy, dosed) ----
            pending_gold = []   # heap-ish FIFO of (min_superstep, closure)
            mem_count = [0]
            mtr_count = [0]
            N_MEM = T
            N_MTR = T - 1
            cur_s = [0]

            def gold_mem(c, k):
                def emit():
                    nc.tensor.matmul(
                        out=m_em[:],
                        lhsT=raw_tiles[c][:, k * K:(k + 1) * K],
                        rhs=oh_pieces[c][:, k * K:(k + 1) * K],
                        start=(mem_count[0] == 0),
                        stop=(mem_count[0] == N_MEM - 1),
                        skip_group_check=True)
                    mem_count[0] += 1
                return emit

            def gold_mtr(t):
                def emit():
                    ca, sa = t // CHUNK_T, t % CHUNK_T
                    cb, sb = (t + 1) // CHUNK_T, (t + 1) % CHUNK_T
                    nc.tensor.matmul(
                        out=m_tr[:],
                        lhsT=oh_pieces[ca][:, sa * K:(sa + 1) * K],
                        rhs=oh_pieces[cb][:, sb * K:(sb + 1) * K],
                        start=(mtr_count[0] == 0),
                        stop=(mtr_count[0] == N_MTR - 1),
                        skip_group_check=True)
                    mtr_count[0] += 1
                return emit

            oh_emit_s = {}

            def prep_oh(c):
                # one-hots are host-built and DMA'd (no legal on-device
                # engine computes is_equal without touching DVE/ACT, whose
                # snapshot semaphores would couple it to the chain).  One
                # tile per chunk: a shared tensor would serialize the last
                # DMAs behind every earlier matmul read of it.
                ohp = constp.tile([BSH, CHUNK_T * K], dt.bfloat16,
                                  tag=f"oh{c}")
                nc.sync.dma_start(
                    out=ohp[:],
                    in_=oh_d[:, c * CHUNK_T * K:(c + 1) * CHUNK_T * K])
                oh_pieces[c] = ohp[:]
                oh_emit_s[c] = cur_s[0]
                ready = cur_s[0] + OH_DELAY
                for k in range(CHUNK_T):
                    pending_gold.append((ready, gold_mem(c, k)))

            def queue_mtr_chunk(c):
                # both one-hots of every t in this range exist already
                ready = max(oh_emit_s.get(c, 0),
                            oh_emit_s.get(c + 1, 0)) + OH_DELAY
                lo = c * CHUNK_T
                hi = min((c + 1) * CHUNK_T, T - 1)
                for t in range(lo, hi):
                    pending_gold.append((ready, gold_mtr(t)))

            def pop_gold(s):
                n = 0
                while n < GOLD_DOSE and pending_gold and pending_gold[0][0] <= s:
                    pending_gold.pop(0)[1]()
                    n += 1

            # ---- pairs exp: slot blocks from the pre-transposed emt ----
            def exp_slots(lo, n):
                nc.scalar.activation(
                    out=pairs[:, lo * 2 * BG:(lo + n) * 2 * BG],
                    in_=emt[:, lo * 2 * BG:(lo + n) * 2 * BG],
                    func=AF.Exp, bias=expbias[:])

            # ---- chain state ----
            st = {"q_rhs": em0[:], "v_rhs": em511[:], "rho": None}
            renorm_idx = [0]

            def superstep(s):
                ps = psPp.tile([128, 2 * BG], dt.float32, tag="qv")
                nc.tensor.matmul(out=ps[:, 0:BG], lhsT=wf, rhs=st["q_rhs"],
                                 start=True, stop=True)
                if s <= 254:
                    nc.tensor.matmul(out=ps[:, BG:2 * BG], lhsT=wb,
                                     rhs=st["v_rhs"], start=True, stop=True)
                if s <= 253:
                    # dedicated buffer per superstep: no pool WAW self-wait,
                    # so the mult's single wait slot holds the PE sem inline
                    qv = constp.tile([128, 2 * BG], dt.bfloat16, tag=f"qv{s}")
                    nc.vector.tensor_tensor(
                        out=qv[:], in0=ps[:],
                        in1=pairs[:, 2 * BG * s:2 * BG * (s + 1)], op=OP.mult)
                    st["q_rhs"] = qv[:, 0:BG]
                    st["v_rhs"] = qv[:, BG:2 * BG]
                    if (s + 1) % NORM_EVERY == 0 and s + 1 < TAU:
                        # off-chain renorm: rescale pairs slot s+12 by
                        # 1/sums; raw sums ship to host for exact log fixup.
                        # Each sub-op is released into a distinct later
                        # window so no chain wait snapshots a long residue.
                        r = renorm_idx[0]
                        renorm_idx[0] += 1
                        s_ps = psNp.tile([G, 2 * BG], dt.float32, tag="s_ps")
                        nc.tensor.matmul(out=s_ps[:], lhsT=onesbd_bf,
                                         rhs=qv[:], start=True, stop=True)
                        nc.vector.tensor_copy(
                            out=out_sb[0:G, OC_SPS + r * 2 * BG:
                                       OC_SPS + (r + 1) * 2 * BG],
                            in_=s_ps[:])
                        rs = miscp.tile([G, 2 * BG], dt.float32, tag="rs")
                        nc.vector.reciprocal(out=rs[:], in_=s_ps[:])
                        bc_ps = psNp.tile([128, 2 * BG], dt.float32, tag="bc_ps")
                        nc.tensor.matmul(out=bc_ps[:], lhsT=onesbc_f,
                                         rhs=rs[:], start=True, stop=True)
                        sl = pairs[:, 2 * BG * (s + 12):2 * BG * (s + 13)]
                        nc.vector.tensor_tensor(out=sl, in0=sl, in1=bc_ps[:],
                                                op=OP.mult)
                else:
                    qv = constp.tile([128, BG], dt.bfloat16, tag=f"qv{s}")
                    nc.vector.tensor_tensor(
                        out=qv[:], in0=ps[:, 0:BG],
                        in1=pairs[:, 2 * BG * s:2 * BG * s + BG], op=OP.mult)
                    st["q_rhs"] = qv[:]
                    if s == 254:
                        st["rho"] = ps

            def emit_gold_finalize():
                # ship the raw m_em / m_tr matrices; the host computes
                # trace(m_em) and sum(m_tr * transitions)
                nc.vector.tensor_copy(out=out_sb[0:K, OC_MEM:OC_MEM + K],
                                      in_=m_em[:])
                nc.vector.tensor_copy(out=out_sb[0:K, OC_MTR:OC_MTR + K],
                                      in_=m_tr[:])

            # ---- head: unblock superstep 0 fast ----
            nc.scalar.activation(out=em0[:], in_=em01[:, 0:BG],
                                 func=AF.Exp, bias=expbias[:])
            nc.scalar.activation(out=em511[:], in_=em01[:, BG:2 * BG],
                                 func=AF.Exp, bias=expbias[:])
            exp_slots(0, 1)
            exp_slots(1, 7)
            exp_slots(8, 24)
            prep_oh(0)
            prep_oh(15)
            queue_mtr_chunk(15)
            nc.sync.dma_start(out=packf[:], in_=packf_d[:])
            # raw chunks for groups 0's golds were loaded at the head; the
            # remaining emt group blocks + raw chunks stream in on SP, one
            # group / chunk-pair ahead of use
            nc.sync.dma_start(out=emt[:, EMT_GRP:2 * EMT_GRP],
                              in_=emt_d[:, EMT_GRP:2 * EMT_GRP])
            raw_pairs = [((i, True), (15 - i, False))
                         for i in range(1, NCHUNKS // 2)]   # chunk pairs 1..7

            def dma_raw(c, fwd_side):
                rawp = rawFp if fwd_side else rawBp
                raw = rawp.tile([128, CHUNK_T * K], dt.bfloat16)
                nc.sync.dma_start(
                    out=raw[:], in_=em_d[:, c * CHUNK_T:(c + 1) * CHUNK_T, :])
                raw_tiles[c] = raw

            dma_raw(*raw_pairs[0][0])
            dma_raw(*raw_pairs[0][1])

            gold_done = [False]
            for h in range(N_GROUPS):
                # prefetch next group's emt block + raw chunk pair
                if h + 2 < N_GROUPS:
                    nc.sync.dma_start(
                        out=emt[:, (h + 2) * EMT_GRP:(h + 3) * EMT_GRP],
                        in_=emt_d[:, (h + 2) * EMT_GRP:(h + 3) * EMT_GRP])
                if h + 1 < len(raw_pairs) + 1 and h + 1 <= 6:
                    dma_raw(*raw_pairs[h + 1][0])
                    dma_raw(*raw_pairs[h + 1][1])
                # one-hots for this group's raw chunk pair
                if h < len(raw_pairs):
                    (cf, _), (cb, _) = raw_pairs[h]
                    prep_oh(cf)
                    prep_oh(cb)
                    queue_mtr_chunk(cf - 1)
                    queue_mtr_chunk(cb)
                    if h == 6:
                        queue_mtr_chunk(7)
                for s in range(h * SS_GROUP, (h + 1) * SS_GROUP):
                    cur_s[0] = s
                    # gold doses: released to the scheduler only around this
                    # superstep's simulated time (emitted before the chain
                    # matmuls so their stream slots precede them) — without
                    # the hold, the scheduler backfills dozens of ready gold
                    # matmuls into every chain-wait window and the lowered
                    # snapshot semaphores bill the chain for all of them
                    with tc.tile_wait_until(_EST(s)):
                        pop_gold(s)
                    superstep(s)
                    # next group's pairs exp, diced to 8-slot pieces emitted
                    # at distinct supersteps
                    if h + 1 < N_GROUPS and s % 8 == 4:
                        exp_slots((h + 1) * SS_GROUP + ((s // 8) % 4) * 8, 8)
            with tc.tile_wait_until(_EST(TAU)):
                while pending_gold:
                    pending_gold.pop(0)[1]()
            # finalize MUST be scheduled after every accumulating matmul:
            # Tile's dep tracker doesn't order PSUM-group readers behind all
            # group members (skip_group_check), so force it via release time
            # — its stream slot then snapshots semaphores past all of them
            with tc.tile_wait_until(_EST(230)):
                emit_gold_finalize()

            # ---- stitch: Z_b = sum_i q_256[i,b] * rho_256[i,b] ----
            u = miscp.tile([128, BG], dt.float32, tag="u")
            nc.vector.tensor_tensor(out=u[:], in0=st["q_rhs"],
                                    in1=st["rho"][:, BG:2 * BG], op=OP.mult)
            z_ps = psNp.tile([G, BG], dt.float32, tag="s_ps")
            nc.tensor.matmul(out=z_ps[:], lhsT=onesbd_f, rhs=u[:],
                             start=True, stop=True)
            nc.vector.tensor_copy(out=out_sb[0:G, OC_Z:OC_Z + BG], in_=z_ps[:])

            nc.sync.dma_start(out=out_d[:], in_=out_sb[:])

    nc.compile()
    return nc


def _host_constants(transitions):
    """Host-prepared constant packs + the exact scale correction."""
    import ml_dtypes
    Tr64 = np.asarray(transitions, dtype=np.float64)
    expT = np.exp(Tr64)
    a = float(np.log(expT.sum() / K))
    Etil = (expT * math.exp(-a)).astype(np.float32)

    wf = np.kron(np.eye(G, dtype=np.float32), Etil)
    wb = np.kron(np.eye(G, dtype=np.float32), Etil.T.copy())
    onesbd = np.kron(np.eye(G, dtype=np.float32), np.ones((K, 1), np.float32))
    onesbc = np.kron(np.eye(G, dtype=np.float32), np.ones((1, K), np.float32))
    iota32 = np.tile(np.arange(K, dtype=np.float32), (128, 1))

    packbf = np.zeros((128, PBF_COLS), dtype=np.float32)
    packbf[:, PBF_WF:PBF_WF + 128] = wf
    packbf[:, PBF_WB:PBF_WB + 128] = wb
    packbf[:, PBF_OBD:PBF_OBD + G] = onesbd
    packbf[:, PBF_IOTA:PBF_IOTA + K] = iota32
    packbf = packbf.astype(ml_dtypes.bfloat16)

    packf = np.zeros((128, PF_COLS), dtype=np.float32)
    packf[:, PF_OBD:PF_OBD + G] = onesbd
    packf[0:G, PF_OBC:PF_OBC + 128] = onesbc
    packf[0:K, PF_EYE:PF_EYE + K] = np.eye(K, dtype=np.float32)
    packf[0:K, PF_TRANS:PF_TRANS + K] = np.asarray(transitions, np.float32)

    corr = (T - 1) * a + T * (-EXP_BIAS)
    return {"packbf": packbf, "packf": packf}, corr


def _host_emt(em_bf):
    """Slot-ordered pre-transposed emissions per core.

    em_bf: [B, T, K] bf16.  Returns (emt [B//BSH stacked 128, EMT_COLS],
    em01 [.., 2*BG]): partition p = 32*g + k, col-block s = 64 cols
    [fwd em_{s+1} (32 batch) | bwd em_{510-s}]; bwd half of slots 254/255
    is zero-filled (never read).
    """
    import ml_dtypes
    # [NCORES, G, BG, T, K] -> [NCORES, G, K, T, BG] (partition = g*K + k)
    X = em_bf.reshape(NCORES, G, BG, T, K).transpose(0, 1, 4, 3, 2)
    X = np.ascontiguousarray(X).reshape(NCORES, 128, T, BG)
    idx_f = np.arange(1, TAU + 1)               # t = 1..256
    idx_b = 510 - np.arange(TAU)                # t = 510..255
    fwd = X[:, :, idx_f, :]                     # [NC, 128, 256, BG]
    bwd = X[:, :, idx_b, :].copy()
    bwd[:, :, 254:, :] = 0
    emt = np.empty((NCORES, 128, TAU, 2 * BG), dtype=em_bf.dtype)
    emt[:, :, :, 0:BG] = fwd
    emt[:, :, :, BG:] = bwd
    emt = emt.reshape(NCORES, 128, EMT_COLS)
    em01 = np.concatenate([X[:, :, 0, :], X[:, :, T - 1, :]], axis=2)
    return emt, em01


def _host_reduce(outs, corr, trans):
    """Combine per-core [K, OUT_COLS] outputs into the scalar loss."""
    total = 0.0
    for o in outs:
        o = np.asarray(o, dtype=np.float64).reshape(K, OUT_COLS)
        logz = np.log(o[0:G, OC_Z:OC_Z + BG])            # [G, BG]
        for r in range(N_RENORM):
            sps = o[0:G, OC_SPS + r * 2 * BG:OC_SPS + (r + 1) * 2 * BG]
            logz = logz + np.log(sps[:, 0:BG]) + np.log(sps[:, BG:2 * BG])
        m_em = o[0:K, OC_MEM:OC_MEM + K]
        m_tr = o[0:K, OC_MTR:OC_MTR + K]
        total += logz.sum() - np.trace(m_em) - (m_tr * trans).sum()
    return total / B + corr


def _host_onehot(tags):
    """Host-built one-hot [B, T*K] bf16: oh[b, t*K+j] = (tags[b,t] == j)."""
    import ml_dtypes
    tags = np.asarray(tags).astype(np.int32)
    oh = (tags[:, :, None] == np.arange(K, dtype=np.int32)[None, None, :])
    return np.ascontiguousarray(
        oh.astype(ml_dtypes.bfloat16).reshape(B, T * K))


def kernel(emissions, tags, mask, transitions):
    import ml_dtypes
    from concourse.bass_utils import run_bass_kernel_spmd

    em_bf = np.ascontiguousarray(
        np.asarray(emissions, dtype=np.float32).astype(ml_dtypes.bfloat16))
    oh = _host_onehot(tags)
    transitions = np.ascontiguousarray(np.asarray(transitions, dtype=np.float32))

    if "nc" not in _PROGRAM_CACHE:
        _PROGRAM_CACHE["nc"] = _build_program()
    nc = _PROGRAM_CACHE["nc"]

    consts, corr = _host_constants(transitions)
    emt, em01 = _host_emt(em_bf)
    core_ids = list(range(NCORES))
    in_maps = []
    for c in core_ids:
        sl = slice(c * BSH, (c + 1) * BSH)
        m = {"emissions": em_bf[sl], "oh": oh[sl],
             "emt": emt[c], "em01": em01[c]}
        m.update(consts)
        in_maps.append(m)

    res = run_bass_kernel_spmd(nc, in_maps, core_ids)
    _PROGRAM_CACHE["last_results"] = res
    loss = _host_reduce([r["out"] for r in res.results], corr,
                        np.asarray(transitions, np.float64))
    return np.float32(loss)
